# revision 8
# baseline (speedup 1.0000x reference)
"""Trainium2 Bass kernel for nn_CausalWanModel (frame-block-causal attention).

Self-contained: hardcodes shapes from the problem spec.
  B=1, T=3120, D=1536, H=12 heads, hd=128, frame_seqlen=780, 8 cores.

Sharding: sequence-parallel. Core c owns tokens [390c, 390c+390) for
q/k/v projections, attention (its queries vs all keys, block-causal mask
via per-key additive bias on the exp), and the o-projection rows. k/v are
exchanged with a single AllGather; the host gathers the 8 row-slices.

Matmuls run in bf16 (fp32 PSUM accumulation); RMSNorm statistics in fp32.
"""

import math

import numpy as np
import ml_dtypes

import concourse.bacc as bacc
import concourse.mybir as mybir
import concourse.tile as tile
from concourse.bass_utils import run_bass_kernel_spmd

F32 = mybir.dt.float32
BF16 = mybir.dt.bfloat16

NC = 8
T = 3120
D = 1536
H = 12
HD = 128
L = 780  # frame_seqlen
CHUNK = T // NC  # 390 tokens per core
KC = D // 128  # 12 contraction chunks
EPS = 1e-6
SCALE = 1.0 / math.sqrt(HD)

KEY_TILES = [(i * 128, min(128, T - i * 128)) for i in range((T + 127) // 128)]
TOK_SUBS = [(0, 128), (128, 128), (256, 128), (384, 6)]
COLG = [(g * 512, 512) for g in range(3)]

K_ELEMS = D * CHUNK
V_ELEMS = CHUNK * D


def build_kernel(apply_bias_qk=False, apply_g=False, apply_bias_v=False,
                 apply_bias_o=False, debug=False):
    nc = bacc.Bacc("TRN2", target_bir_lowering=False, debug=False, num_devices=NC)

    # ---- I/O ----
    xT = nc.dram_tensor("xT", [D, CHUNK], BF16, kind="ExternalInput")
    wq = nc.dram_tensor("wq", [D, D], BF16, kind="ExternalInput")
    wk = nc.dram_tensor("wk", [D, D], BF16, kind="ExternalInput")
    wv = nc.dram_tensor("wv", [D, D], BF16, kind="ExternalInput")
    wo = nc.dram_tensor("wo", [D, D], BF16, kind="ExternalInput")
    cost = nc.dram_tensor("cost", [128, CHUNK], F32, kind="ExternalInput")
    sint = nc.dram_tensor("sint", [128, CHUNK], F32, kind="ExternalInput")
    maskv = nc.dram_tensor("maskv", [T], F32, kind="ExternalInput")
    bqk2 = nc.dram_tensor("bqk2", [2 * KC, 128], F32, kind="ExternalInput")
    gqk2 = nc.dram_tensor("gqk2", [2 * KC, 128], F32, kind="ExternalInput")
    bvo = nc.dram_tensor("bvo", [2, D], F32, kind="ExternalInput")
    out_part = nc.dram_tensor("out_part", [CHUNK, D], F32, kind="ExternalOutput")

    # ---- collective buffers ----
    kv_in = nc.dram_tensor("kv_in", [K_ELEMS + V_ELEMS], BF16)
    kv_out = nc.dram_tensor("kv_out", [NC, K_ELEMS + V_ELEMS], BF16,
                            addr_space="Shared")
    v_flat = nc.dram_tensor("v_flat", [T, D], BF16)

    if debug:
        dbg_qT = nc.dram_tensor("dbg_qT", [128, KC * CHUNK], F32, kind="ExternalOutput")
        dbg_kT = nc.dram_tensor("dbg_kT", [128, KC * CHUNK], F32, kind="ExternalOutput")
        dbg_sums = nc.dram_tensor("dbg_sums", [H, CHUNK], F32, kind="ExternalOutput")
        dbg_attnT = nc.dram_tensor("dbg_attnT", [128, KC * CHUNK], F32,
                                   kind="ExternalOutput")

    kT_view = kv_in.ap()[0:K_ELEMS].rearrange("(r t) -> r t", t=CHUNK)
    v_view = kv_in.ap()[K_ELEMS:].rearrange("(t c) -> t c", c=D)

    with tile.TileContext(nc) as tc:
        with tc.tile_pool(name="const", bufs=1) as cpool:
            xT_sb = cpool.tile([128, KC * CHUNK], BF16, tag="xT_sb")
            qT_sb = cpool.tile([128, KC * CHUNK], BF16, tag="qT_sb")
            attnT_sb = cpool.tile([128, KC * CHUNK], BF16, tag="attnT_sb")
            cost_sb = cpool.tile([128, CHUNK], F32, tag="cost_sb")
            sint_sb = cpool.tile([128, CHUNK], F32, tag="sint_sb")
            masks_sb = cpool.tile([128, len(KEY_TILES)], F32, tag="masks_sb")
            ones_f32 = cpool.tile([128, 1], F32, tag="ones_f32")
            ones_bf = cpool.tile([128, 1], BF16, tag="ones_bf")
            sq_scale = cpool.tile([1, CHUNK], F32, tag="sq_scale")
            sk_scale = cpool.tile([1, CHUNK], F32, tag="sk_scale")
            sq_bc = cpool.tile([128, CHUNK], F32, tag="sq_bc")
            sk_bc = cpool.tile([128, CHUNK], F32, tag="sk_bc")
            u_tiles = {(name, d): cpool.tile([128, CHUNK], F32,
                                             name=f"u_{name}_{d}",
                                             tag=f"u_{name}_{d}")
                       for name in ("q", "k") for d in range(KC)}

            eps_sb = cpool.tile([1, 1], F32, tag="eps_sb")
            nc.gpsimd.memset(ones_f32[:, :], 1.0)
            nc.gpsimd.memset(ones_bf[:, :], 1.0)
            nc.gpsimd.memset(eps_sb[:, :], EPS)

            for d in range(KC):
                nc.sync.dma_start(out=xT_sb[:, d * CHUNK:(d + 1) * CHUNK],
                                  in_=xT[d * 128:(d + 1) * 128, :])
            nc.sync.dma_start(out=cost_sb[:, :], in_=cost[:, :])
            nc.sync.dma_start(out=sint_sb[:, :], in_=sint[:, :])
            nc.sync.dma_start(
                out=masks_sb[:, 0:24],
                in_=maskv.ap()[0:3072].rearrange("(t p) -> p t", p=128))
            nc.sync.dma_start(
                out=masks_sb[0:48, 24:25],
                in_=maskv.ap()[3072:3120].rearrange("(t p) -> p t", p=48))
            bqk_sb = gqk_sb = bvo_sb = None
            if apply_bias_qk:
                bqk_sb = cpool.tile([128, 2 * KC], F32, tag="bqk_sb")
                nc.sync.dma_start(out=bqk_sb[:, :],
                                  in_=bqk2.ap().rearrange("c p -> p c"))
            if apply_g:
                gqk_sb = cpool.tile([128, 2 * KC], F32, tag="gqk_sb")
                nc.sync.dma_start(out=gqk_sb[:, :],
                                  in_=gqk2.ap().rearrange("c p -> p c"))
            if apply_bias_v or apply_bias_o:
                bvo_sb = cpool.tile([2, D], F32, tag="bvo_sb")
                nc.sync.dma_start(out=bvo_sb[:, :], in_=bvo[:, :])

            # =========== Phase 1: projections + rmsnorm + rope ===========
            with tc.tile_pool(name="p1sb", bufs=3) as p1sb, \
                 tc.tile_pool(name="p1w", bufs=3) as p1w, \
                 tc.tile_pool(name="p1ps", bufs=2, space="PSUM") as p1ps, \
                 tc.tile_pool(name="ssqps", bufs=1, space="PSUM") as ssqps:

                ssq_ps = {}
                for name, w, is_q in (("q", wq, True), ("k", wk, False)):
                    ssq_ps[name] = ssqps.tile([1, CHUNK], F32, name=f"ssq_{name}", tag=f"ssq_{name}")
                    for d in range(KC):
                        wt = p1w.tile([128, D], BF16, tag="wqk_t")
                        nc.sync.dma_start(
                            out=wt[:, :].rearrange("p (c m) -> p c m", c=KC),
                            in_=w[0:D, d * 128:(d + 1) * 128]
                            .rearrange("(c p) m -> p c m", p=128))
                        ps = p1ps.tile([128, CHUNK], F32, tag="proj_ps")
                        for c in range(KC):
                            nc.tensor.matmul(
                                ps[:, :],
                                wt[:, c * 128:(c + 1) * 128],
                                xT_sb[:, c * CHUNK:(c + 1) * CHUNK],
                                start=(c == 0), stop=(c == KC - 1))
                        ur = u_tiles[(name, d)]
                        if apply_bias_qk:
                            bias_col = (0 if is_q else KC) + d
                            nc.vector.tensor_scalar_add(
                                ur[:, :], ps[:, :], bqk_sb[:, bias_col:bias_col + 1])
                        else:
                            nc.vector.tensor_copy(ur[:, :], ps[:, :])
                        sq = p1sb.tile([128, CHUNK], F32, tag="sqsb")
                        nc.vector.tensor_tensor(sq[:, :], ur[:, :], ur[:, :],
                                                mybir.AluOpType.mult)
                        nc.tensor.matmul(ssq_ps[name][:, :], ones_f32[:, :], sq[:, :],
                                         start=(d == 0), stop=(d == KC - 1))

                for name, stile, sbc in (("q", sq_scale, sq_bc),
                                         ("k", sk_scale, sk_bc)):
                    nc.scalar.activation(stile[:, :], ssq_ps[name][:, :],
                                         mybir.ActivationFunctionType.Sqrt,
                                         bias=eps_sb[:, :], scale=1.0 / D)
                    nc.vector.reciprocal(stile[:, :], stile[:, :])
                    nc.gpsimd.partition_broadcast(sbc[:, :], stile[:, :])

                for name in ("q", "k"):
                    sbc = sq_bc if name == "q" else sk_bc
                    for d in range(KC):
                        ur = u_tiles[(name, d)]
                        qs = p1sb.tile([128, CHUNK], F32, tag="qs")
                        nc.vector.tensor_tensor(
                            qs[:, :], ur[:, :], sbc[:, :],
                            mybir.AluOpType.mult)
                        if apply_g:
                            gcol = (0 if name == "q" else KC) + d
                            nc.vector.tensor_scalar_mul(
                                qs[:, :], qs[:, :], gqk_sb[:, gcol:gcol + 1])
                        qsw = p1sb.tile([128, CHUNK], F32, tag="qsw")
                        nc.sync.dma_start(out=qsw[0:64, :], in_=qs[64:128, :])
                        nc.sync.dma_start(out=qsw[64:128, :], in_=qs[0:64, :])
                        t1 = p1sb.tile([128, CHUNK], F32, tag="rope_t1")
                        t2 = p1sb.tile([128, CHUNK], F32, tag="rope_t2")
                        nc.vector.tensor_tensor(t1[:, :], qs[:, :], cost_sb[:, :],
                                                mybir.AluOpType.mult)
                        nc.vector.tensor_tensor(t2[:, :], qsw[:, :], sint_sb[:, :],
                                                mybir.AluOpType.mult)
                        if name == "q":
                            dst = qT_sb[:, d * CHUNK:(d + 1) * CHUNK]
                            nc.vector.tensor_tensor(dst[:, :], t1[:, :], t2[:, :],
                                                    mybir.AluOpType.add)
                            if debug:
                                df = p1sb.tile([128, CHUNK], F32, tag="dbgf")
                                nc.vector.tensor_copy(df[:, :], dst)
                                nc.sync.dma_start(
                                    out=dbg_qT[:, d * CHUNK:(d + 1) * CHUNK],
                                    in_=df[:, :])
                        else:
                            kr = p1sb.tile([128, CHUNK], BF16, tag="krope")
                            nc.vector.tensor_tensor(kr[:, :], t1[:, :], t2[:, :],
                                                    mybir.AluOpType.add)
                            nc.sync.dma_start(
                                out=kT_view[d * 128:(d + 1) * 128, :],
                                in_=kr[:, :])
                            if debug:
                                df = p1sb.tile([128, CHUNK], F32, tag="dbgf")
                                nc.vector.tensor_copy(df[:, :], kr[:, :])
                                nc.sync.dma_start(
                                    out=dbg_kT[:, d * CHUNK:(d + 1) * CHUNK],
                                    in_=df[:, :])

                # v projection
                for (c0, csz) in COLG:
                    wt = p1w.tile([128, KC * 512], BF16, tag="wv_t")
                    nc.sync.dma_start(
                        out=wt[:, :].rearrange("p (c m) -> p c m", c=KC),
                        in_=wv[0:D, c0:c0 + csz]
                        .rearrange("(c p) m -> p c m", p=128))
                    for (t0, tsz) in TOK_SUBS:
                        ps = p1ps.tile([128, 512], F32, tag="v_ps")
                        for c in range(KC):
                            nc.tensor.matmul(
                                ps[0:tsz, :],
                                xT_sb[:, c * CHUNK + t0:c * CHUNK + t0 + tsz],
                                wt[:, c * 512:(c + 1) * 512],
                                start=(c == 0), stop=(c == KC - 1))
                        vsb = p1sb.tile([128, 512], BF16, tag="vsb")
                        if apply_bias_v:
                            bvb = p1sb.tile([128, 512], F32, tag="bvb")
                            nc.gpsimd.partition_broadcast(
                                bvb[:, :], bvo_sb[0:1, c0:c0 + csz])
                            nc.vector.tensor_tensor(
                                vsb[0:tsz, :], ps[0:tsz, :], bvb[0:tsz, :],
                                mybir.AluOpType.add)
                        else:
                            nc.vector.tensor_copy(vsb[0:tsz, :], ps[0:tsz, :])
                        nc.sync.dma_start(out=v_view[t0:t0 + tsz, c0:c0 + csz],
                                          in_=vsb[0:tsz, :])

            # =========== Phase 2: AllGather k/v ===========
            nc.gpsimd.collective_compute(
                "AllGather", mybir.AluOpType.bypass,
                ins=[kv_in.ap().opt()],
                outs=[kv_out.ap().opt()],
                replica_groups=[list(range(NC))],
            )
            for r in range(NC):
                nc.sync.dma_start(
                    out=v_flat[r * CHUNK:(r + 1) * CHUNK, :],
                    in_=kv_out.ap()[r, K_ELEMS:].rearrange("(t c) -> t c", c=D))

            # =========== Phase 3: attention ===========
            NKT = len(KEY_TILES)
            with tc.tile_pool(name="a_k", bufs=2) as akp, \
                 tc.tile_pool(name="a_v", bufs=2) as avp, \
                 tc.tile_pool(name="a_p", bufs=4) as app, \
                 tc.tile_pool(name="a_sb", bufs=3) as asb, \
                 tc.tile_pool(name="a_ps", bufs=3, space="PSUM") as aps, \
                 tc.tile_pool(name="acc_ps", bufs=2, space="PSUM") as accps, \
                 tc.tile_pool(name="sum_ps", bufs=2, space="PSUM") as sumps:
                for h in range(H):
                    kt_sb = akp.tile([128, T], BF16, tag="kt_sb")
                    for r in range(NC):
                        nc.sync.dma_start(
                            out=kt_sb[:, r * CHUNK:(r + 1) * CHUNK],
                            in_=kv_out.ap()[r, 0:K_ELEMS]
                            .rearrange("(row t) -> row t", t=CHUNK)
                            [h * 128:(h + 1) * 128, :])
                    vt_sb = avp.tile([128, NKT, 128], BF16, tag="vt_sb")
                    nc.sync.dma_start(
                        out=vt_sb[:, 0:24, :],
                        in_=v_flat.ap()[0:3072, h * 128:(h + 1) * 128]
                        .rearrange("(t p) c -> p t c", p=128))
                    nc.sync.dma_start(
                        out=vt_sb[0:48, 24:25, :],
                        in_=v_flat.ap()[3072:3120, h * 128:(h + 1) * 128]
                        .rearrange("(t p) c -> p t c", p=48))

                    acc = accps.tile([128, CHUNK], F32, tag="acc")
                    sums = sumps.tile([1, CHUNK], F32, tag="sums")
                    for kt, (k0, ksz) in enumerate(KEY_TILES):
                        sc = aps.tile([128, CHUNK], F32, tag="sc")
                        nc.tensor.matmul(
                            sc[0:ksz, :],
                            kt_sb[:, k0:k0 + ksz],
                            qT_sb[:, h * CHUNK:(h + 1) * CHUNK],
                            start=True, stop=True)
                        pr = app.tile([128, CHUNK], BF16, tag="pr")
                        nc.scalar.activation(
                            pr[0:ksz, :], sc[0:ksz, :],
                            mybir.ActivationFunctionType.Exp,
                            bias=masks_sb[0:ksz, kt:kt + 1], scale=SCALE)
                        nc.tensor.matmul(
                            acc[:, :],
                            vt_sb[0:ksz, kt, :],
                            pr[0:ksz, :],
                            start=(kt == 0), stop=(kt == NKT - 1))
                        nc.tensor.matmul(
                            sums[:, :],
                            ones_bf[0:ksz, :],
                            pr[0:ksz, :],
                            start=(kt == 0), stop=(kt == NKT - 1))
                    rec = asb.tile([1, CHUNK], F32, tag="rec")
                    nc.vector.reciprocal(rec[:, :], sums[:, :])
                    recb = asb.tile([128, CHUNK], F32, tag="recb")
                    nc.gpsimd.partition_broadcast(recb[:, :], rec[:, :])
                    nc.vector.tensor_tensor(
                        attnT_sb[:, h * CHUNK:(h + 1) * CHUNK],
                        acc[:, :], recb[:, :],
                        mybir.AluOpType.mult)
                    if debug:
                        ssb = asb.tile([1, CHUNK], F32, tag="ssb")
                        nc.vector.tensor_copy(ssb[:, :], sums[:, :])
                        nc.sync.dma_start(out=dbg_sums[h:h + 1, :], in_=ssb[:, :])
                        da = asb.tile([128, CHUNK], F32, tag="da")
                        nc.vector.tensor_copy(
                            da[:, :], attnT_sb[:, h * CHUNK:(h + 1) * CHUNK])
                        nc.sync.dma_start(
                            out=dbg_attnT[:, h * CHUNK:(h + 1) * CHUNK],
                            in_=da[:, :])

            # =========== Phase 4: o-projection ===========
            with tc.tile_pool(name="p4sb", bufs=3) as p4sb, \
                 tc.tile_pool(name="p4w", bufs=2) as p4w, \
                 tc.tile_pool(name="p4ps", bufs=3, space="PSUM") as p4ps:
                for (c0, csz) in COLG:
                    wt = p4w.tile([128, KC * 512], BF16, tag="wo_t")
                    nc.sync.dma_start(
                        out=wt[:, :].rearrange("p (c m) -> p c m", c=KC),
                        in_=wo[0:D, c0:c0 + csz]
                        .rearrange("(c p) m -> p c m", p=128))
                    for (t0, tsz) in TOK_SUBS:
                        ps = p4ps.tile([128, 512], F32, tag="o_ps")
                        for hh in range(KC):
                            nc.tensor.matmul(
                                ps[0:tsz, :],
                                attnT_sb[:, hh * CHUNK + t0:hh * CHUNK + t0 + tsz],
                                wt[:, hh * 512:(hh + 1) * 512],
                                start=(hh == 0), stop=(hh == KC - 1))
                        osb = p4sb.tile([128, 512], F32, tag="osb")
                        if apply_bias_o:
                            bob = p4sb.tile([128, 512], F32, tag="bob")
                            nc.gpsimd.partition_broadcast(
                                bob[:, :], bvo_sb[1:2, c0:c0 + csz])
                            nc.vector.tensor_tensor(
                                osb[0:tsz, :], ps[0:tsz, :], bob[0:tsz, :],
                                mybir.AluOpType.add)
                        else:
                            nc.vector.tensor_copy(osb[0:tsz, :], ps[0:tsz, :])
                        nc.sync.dma_start(out=out_part[t0:t0 + tsz, c0:c0 + csz],
                                          in_=osb[0:tsz, :])

    nc.compile()
    return nc


_NC_CACHE = {}


def _get_nc(key):
    if key not in _NC_CACHE:
        _NC_CACHE[key] = build_kernel(*key)
    return _NC_CACHE[key]


def _prep_inputs(x, freqs_cos, freqs_sin, Wq, bq, Wk, bk, Wv, bv, Wo, bo,
                 gq, gk, frame_seqlen, debug=False):
    assert int(frame_seqlen) == L
    x2d = np.asarray(x, np.float32).reshape(T, D)
    xT_full = np.ascontiguousarray(x2d.T)

    perm = np.concatenate([
        np.concatenate([np.arange(0, 128, 2), np.arange(1, 128, 2)]) + 128 * h
        for h in range(H)])
    Wqp = np.asarray(Wq, np.float32)[:, perm]
    Wkp = np.asarray(Wk, np.float32)[:, perm]
    bqp = np.asarray(bq, np.float32)[perm]
    bkp = np.asarray(bk, np.float32)[perm]
    gqp = np.asarray(gq, np.float32)[perm]
    gkp = np.asarray(gk, np.float32)[perm]

    cosT = np.asarray(freqs_cos, np.float32).T
    sinT = np.asarray(freqs_sin, np.float32).T
    costab = np.concatenate([cosT, cosT], 0)
    sintab = np.concatenate([-sinT, sinT], 0)

    frames = np.arange(T) // L
    bf16 = ml_dtypes.bfloat16

    apply_bias_qk = not (np.all(bqp == 0) and np.all(bkp == 0))
    apply_g = not (np.all(gqp == 1) and np.all(gkp == 1))
    apply_bias_v = not np.all(np.asarray(bv) == 0)
    apply_bias_o = not np.all(np.asarray(bo) == 0)
    key = (apply_bias_qk, apply_g, apply_bias_v, apply_bias_o, debug)

    shared = {
        "wq": Wqp.astype(bf16), "wk": Wkp.astype(bf16),
        "wv": np.asarray(Wv, np.float32).astype(bf16),
        "wo": np.asarray(Wo, np.float32).astype(bf16),
        "bqk2": np.concatenate([bqp, bkp]).reshape(2 * KC, 128),
        "gqk2": np.concatenate([gqp, gkp]).reshape(2 * KC, 128),
        "bvo": np.stack([np.asarray(bv, np.float32),
                         np.asarray(bo, np.float32)]),
    }
    in_maps = []
    for c in range(NC):
        t0 = c * CHUNK
        f_c = t0 // L
        m = np.where(frames <= f_c, 0.0, -30000.0).astype(np.float32)
        in_maps.append({
            **shared,
            "xT": np.ascontiguousarray(xT_full[:, t0:t0 + CHUNK]).astype(bf16),
            "cost": np.ascontiguousarray(costab[:, t0:t0 + CHUNK]),
            "sint": np.ascontiguousarray(sintab[:, t0:t0 + CHUNK]),
            "maskv": m,
        })
    return key, in_maps


def kernel(x, freqs_cos, freqs_sin, Wq, bq, Wk, bk, Wv, bv, Wo, bo,
           gq, gk, frame_seqlen):
    key, in_maps = _prep_inputs(x, freqs_cos, freqs_sin, Wq, bq, Wk, bk,
                                Wv, bv, Wo, bo, gq, gk, frame_seqlen)
    nc = _get_nc(key)
    res = run_bass_kernel_spmd(nc, in_maps, core_ids=list(range(NC)))
    out = np.empty((1, T, D), np.float32)
    for c in range(NC):
        out[0, c * CHUNK:(c + 1) * CHUNK, :] = res.results[c]["out_part"]
    return out


# revision 11
# speedup vs baseline: 1.0558x; 1.0558x over previous
"""Trainium2 Bass kernel for nn_CausalWanModel (frame-block-causal attention).

Self-contained: hardcodes shapes from the problem spec.
  B=1, T=3120, D=1536, H=12 heads, hd=128, frame_seqlen=780, 8 cores.

Sharding: sequence-parallel. Core c owns tokens [390c, 390c+390) for
q/k/v projections, attention (its queries vs all keys, block-causal mask
via per-key additive bias on the exp), and the o-projection rows. k/v are
exchanged with a single AllGather; the host gathers the 8 row-slices.

Matmuls run in bf16 (fp32 PSUM accumulation); RMSNorm statistics in fp32.
"""

import math

import numpy as np
import ml_dtypes

import concourse.bacc as bacc
import concourse.mybir as mybir
import concourse.tile as tile
from concourse.bass_utils import run_bass_kernel_spmd

F32 = mybir.dt.float32
BF16 = mybir.dt.bfloat16

NC = 8
T = 3120
D = 1536
H = 12
HD = 128
L = 780  # frame_seqlen
CHUNK = T // NC  # 390 tokens per core
KC = D // 128  # 12 contraction chunks
EPS = 1e-6
SCALE = 1.0 / math.sqrt(HD)

KEY_TILES = [(i * 128, min(128, T - i * 128)) for i in range((T + 127) // 128)]
TOK_SUBS = [(0, 128), (128, 128), (256, 128), (384, 6)]
COLG = [(g * 512, 512) for g in range(3)]

K_ELEMS = D * CHUNK
V_ELEMS = CHUNK * D


def build_kernel(apply_bias_qk=False, apply_g=False, apply_bias_v=False,
                 apply_bias_o=False, debug=False):
    nc = bacc.Bacc("TRN2", target_bir_lowering=False, debug=False, num_devices=NC)

    # ---- I/O ----
    xT = nc.dram_tensor("xT", [D, CHUNK], BF16, kind="ExternalInput")
    wq = nc.dram_tensor("wq", [D, D], BF16, kind="ExternalInput")
    wk = nc.dram_tensor("wk", [D, D], BF16, kind="ExternalInput")
    wv = nc.dram_tensor("wv", [D, D], BF16, kind="ExternalInput")
    wo = nc.dram_tensor("wo", [D, D], BF16, kind="ExternalInput")
    cost = nc.dram_tensor("cost", [128, CHUNK], F32, kind="ExternalInput")
    sint = nc.dram_tensor("sint", [128, CHUNK], F32, kind="ExternalInput")
    maskv = nc.dram_tensor("maskv", [T], F32, kind="ExternalInput")
    bqk2 = nc.dram_tensor("bqk2", [2 * KC, 128], F32, kind="ExternalInput")
    gqk2 = nc.dram_tensor("gqk2", [2 * KC, 128], F32, kind="ExternalInput")
    bvo = nc.dram_tensor("bvo", [2, D], F32, kind="ExternalInput")
    out_part = nc.dram_tensor("out_part", [CHUNK, D], F32, kind="ExternalOutput")

    # ---- collective buffers ----
    k_in = nc.dram_tensor("k_in", [K_ELEMS], BF16)
    v_in = nc.dram_tensor("v_in", [V_ELEMS], BF16)
    k_out = nc.dram_tensor("k_out", [NC, K_ELEMS], BF16, addr_space="Shared")
    v_out = nc.dram_tensor("v_out", [NC, V_ELEMS], BF16, addr_space="Shared")
    v_flat = nc.dram_tensor("v_flat", [T, D], BF16)

    if debug:
        dbg_qT = nc.dram_tensor("dbg_qT", [128, KC * CHUNK], F32, kind="ExternalOutput")
        dbg_kT = nc.dram_tensor("dbg_kT", [128, KC * CHUNK], F32, kind="ExternalOutput")
        dbg_sums = nc.dram_tensor("dbg_sums", [H, CHUNK], F32, kind="ExternalOutput")
        dbg_attnT = nc.dram_tensor("dbg_attnT", [128, KC * CHUNK], F32,
                                   kind="ExternalOutput")

    kT_view = k_in.ap().rearrange("(r t) -> r t", t=CHUNK)
    v_view = v_in.ap().rearrange("(t c) -> t c", c=D)

    with tile.TileContext(nc) as tc:
        with tc.tile_pool(name="const", bufs=1) as cpool:
            xT_sb = cpool.tile([128, KC * CHUNK], BF16, tag="xT_sb")
            qT_sb = cpool.tile([128, KC * CHUNK], BF16, tag="qT_sb")
            attnT_sb = cpool.tile([128, KC * CHUNK], BF16, tag="attnT_sb")
            cost_sb = cpool.tile([128, CHUNK], F32, tag="cost_sb")
            sint_sb = cpool.tile([128, CHUNK], F32, tag="sint_sb")
            masks_sb = cpool.tile([128, len(KEY_TILES)], F32, tag="masks_sb")
            ones_f32 = cpool.tile([128, 1], F32, tag="ones_f32")
            ones_bf = cpool.tile([128, 1], BF16, tag="ones_bf")
            sq_scale = cpool.tile([1, CHUNK], F32, tag="sq_scale")
            sk_scale = cpool.tile([1, CHUNK], F32, tag="sk_scale")
            sq_bc = cpool.tile([128, CHUNK], F32, tag="sq_bc")
            sk_bc = cpool.tile([128, CHUNK], F32, tag="sk_bc")
            u_tiles = {(name, d): cpool.tile([128, CHUNK], F32,
                                             name=f"u_{name}_{d}",
                                             tag=f"u_{name}_{d}")
                       for name in ("q", "k") for d in range(KC)}

            eps_sb = cpool.tile([1, 1], F32, tag="eps_sb")
            nc.gpsimd.memset(ones_f32[:, :], 1.0)
            nc.gpsimd.memset(ones_bf[:, :], 1.0)
            nc.gpsimd.memset(eps_sb[:, :], EPS)

            for d in range(KC):
                nc.sync.dma_start(out=xT_sb[:, d * CHUNK:(d + 1) * CHUNK],
                                  in_=xT[d * 128:(d + 1) * 128, :])
            nc.sync.dma_start(out=cost_sb[:, :], in_=cost[:, :])
            nc.sync.dma_start(out=sint_sb[:, :], in_=sint[:, :])
            nc.sync.dma_start(
                out=masks_sb[:, 0:24],
                in_=maskv.ap()[0:3072].rearrange("(t p) -> p t", p=128))
            nc.sync.dma_start(
                out=masks_sb[0:48, 24:25],
                in_=maskv.ap()[3072:3120].rearrange("(t p) -> p t", p=48))
            bqk_sb = gqk_sb = bvo_sb = None
            if apply_bias_qk:
                bqk_sb = cpool.tile([128, 2 * KC], F32, tag="bqk_sb")
                nc.sync.dma_start(out=bqk_sb[:, :],
                                  in_=bqk2.ap().rearrange("c p -> p c"))
            if apply_g:
                gqk_sb = cpool.tile([128, 2 * KC], F32, tag="gqk_sb")
                nc.sync.dma_start(out=gqk_sb[:, :],
                                  in_=gqk2.ap().rearrange("c p -> p c"))
            if apply_bias_v or apply_bias_o:
                bvo_sb = cpool.tile([2, D], F32, tag="bvo_sb")
                nc.sync.dma_start(out=bvo_sb[:, :], in_=bvo[:, :])

            # ===== Phase 1: projections + rmsnorm + rope (k -> AG_k -> v -> AG_v -> q) =====
            with tc.tile_pool(name="p1sb", bufs=3) as p1sb, \
                 tc.tile_pool(name="p1w", bufs=3) as p1w, \
                 tc.tile_pool(name="p1ps", bufs=2, space="PSUM") as p1ps, \
                 tc.tile_pool(name="ssqps", bufs=1, space="PSUM") as ssqps:

                ssq_ps = {}

                def qk_proj(name, w, is_q):
                    ssq_ps[name] = ssqps.tile([1, CHUNK], F32, name=f"ssq_{name}",
                                              tag=f"ssq_{name}")
                    for d in range(KC):
                        wt = p1w.tile([128, D], BF16, tag="wqk_t", name="wqk_t")
                        nc.sync.dma_start(
                            out=wt[:, :].rearrange("p (c m) -> p c m", c=KC),
                            in_=w[0:D, d * 128:(d + 1) * 128]
                            .rearrange("(c p) m -> p c m", p=128))
                        ps = p1ps.tile([128, CHUNK], F32, tag="proj_ps",
                                       name="proj_ps")
                        for c in range(KC):
                            nc.tensor.matmul(
                                ps[:, :],
                                wt[:, c * 128:(c + 1) * 128],
                                xT_sb[:, c * CHUNK:(c + 1) * CHUNK],
                                start=(c == 0), stop=(c == KC - 1))
                        ur = u_tiles[(name, d)]
                        if apply_bias_qk:
                            bias_col = (0 if is_q else KC) + d
                            nc.vector.tensor_scalar_add(
                                ur[:, :], ps[:, :], bqk_sb[:, bias_col:bias_col + 1])
                        else:
                            nc.vector.tensor_copy(ur[:, :], ps[:, :])
                        sq = p1sb.tile([128, CHUNK], BF16, tag="sqsb", name="sqsb")
                        nc.vector.tensor_tensor(sq[:, :], ur[:, :], ur[:, :],
                                                mybir.AluOpType.mult)
                        nc.tensor.matmul(ssq_ps[name][:, :], ones_bf[:, :], sq[:, :],
                                         start=(d == 0), stop=(d == KC - 1))

                def qk_scales(name, stile, sbc):
                    nc.scalar.activation(stile[:, :], ssq_ps[name][:, :],
                                         mybir.ActivationFunctionType.Sqrt,
                                         bias=eps_sb[:, :], scale=1.0 / D)
                    nc.vector.reciprocal(stile[:, :], stile[:, :])
                    nc.gpsimd.partition_broadcast(sbc[:, :], stile[:, :])

                def qk_rope(name, sbc):
                    for d in range(KC):
                        ur = u_tiles[(name, d)]
                        qs = p1sb.tile([128, CHUNK], F32, tag="qs", name="qs")
                        nc.vector.tensor_tensor(
                            qs[:, :], ur[:, :], sbc[:, :],
                            mybir.AluOpType.mult)
                        if apply_g:
                            gcol = (0 if name == "q" else KC) + d
                            nc.vector.tensor_scalar_mul(
                                qs[:, :], qs[:, :], gqk_sb[:, gcol:gcol + 1])
                        qsw = p1sb.tile([128, CHUNK], F32, tag="qsw", name="qsw")
                        nc.sync.dma_start(out=qsw[0:64, :], in_=qs[64:128, :])
                        nc.sync.dma_start(out=qsw[64:128, :], in_=qs[0:64, :])
                        t1 = p1sb.tile([128, CHUNK], F32, tag="rope_t1", name="rope_t1")
                        t2 = p1sb.tile([128, CHUNK], F32, tag="rope_t2", name="rope_t2")
                        nc.vector.tensor_tensor(t1[:, :], qs[:, :], cost_sb[:, :],
                                                mybir.AluOpType.mult)
                        nc.vector.tensor_tensor(t2[:, :], qsw[:, :], sint_sb[:, :],
                                                mybir.AluOpType.mult)
                        if name == "q":
                            dst = qT_sb[:, d * CHUNK:(d + 1) * CHUNK]
                            nc.vector.tensor_tensor(dst[:, :], t1[:, :], t2[:, :],
                                                    mybir.AluOpType.add)
                            if debug:
                                df = p1sb.tile([128, CHUNK], F32, tag="dbgf",
                                               name="dbgf")
                                nc.vector.tensor_copy(df[:, :], dst)
                                nc.sync.dma_start(
                                    out=dbg_qT[:, d * CHUNK:(d + 1) * CHUNK],
                                    in_=df[:, :])
                        else:
                            kr = p1sb.tile([128, CHUNK], BF16, tag="krope",
                                           name="krope")
                            nc.vector.tensor_tensor(kr[:, :], t1[:, :], t2[:, :],
                                                    mybir.AluOpType.add)
                            nc.sync.dma_start(
                                out=kT_view[d * 128:(d + 1) * 128, :],
                                in_=kr[:, :])
                            if debug:
                                df = p1sb.tile([128, CHUNK], F32, tag="dbgf",
                                               name="dbgf")
                                nc.vector.tensor_copy(df[:, :], kr[:, :])
                                nc.sync.dma_start(
                                    out=dbg_kT[:, d * CHUNK:(d + 1) * CHUNK],
                                    in_=df[:, :])

                # ---- k first, then AG_k ----
                qk_proj("k", wk, False)
                qk_scales("k", sk_scale, sk_bc)
                qk_rope("k", sk_bc)
                nc.gpsimd.collective_compute(
                    "AllGather", mybir.AluOpType.bypass,
                    ins=[k_in.ap().opt()],
                    outs=[k_out.ap().opt()],
                    replica_groups=[list(range(NC))],
                )

                # ---- v projection, then AG_v ----
                for (c0, csz) in COLG:
                    wt = p1w.tile([128, KC * 512], BF16, tag="wv_t", name="wv_t")
                    nc.sync.dma_start(
                        out=wt[:, :].rearrange("p (c m) -> p c m", c=KC),
                        in_=wv[0:D, c0:c0 + csz]
                        .rearrange("(c p) m -> p c m", p=128))
                    for (t0, tsz) in TOK_SUBS:
                        ps = p1ps.tile([128, 512], F32, tag="v_ps", name="v_ps")
                        for c in range(KC):
                            nc.tensor.matmul(
                                ps[0:tsz, :],
                                xT_sb[:, c * CHUNK + t0:c * CHUNK + t0 + tsz],
                                wt[:, c * 512:(c + 1) * 512],
                                start=(c == 0), stop=(c == KC - 1))
                        vsb = p1sb.tile([128, 512], BF16, tag="vsb", name="vsb")
                        if apply_bias_v:
                            bvb = p1sb.tile([128, 512], F32, tag="bvb", name="bvb")
                            nc.gpsimd.partition_broadcast(
                                bvb[:, :], bvo_sb[0:1, c0:c0 + csz])
                            nc.vector.tensor_tensor(
                                vsb[0:tsz, :], ps[0:tsz, :], bvb[0:tsz, :],
                                mybir.AluOpType.add)
                        else:
                            nc.vector.tensor_copy(vsb[0:tsz, :], ps[0:tsz, :])
                        nc.sync.dma_start(out=v_view[t0:t0 + tsz, c0:c0 + csz],
                                          in_=vsb[0:tsz, :])
                nc.gpsimd.collective_compute(
                    "AllGather", mybir.AluOpType.bypass,
                    ins=[v_in.ap().opt()],
                    outs=[v_out.ap().opt()],
                    replica_groups=[list(range(NC))],
                )
                repack_engines = [nc.sync, nc.scalar, nc.gpsimd]
                for r in range(NC):
                    repack_engines[r % 3].dma_start(
                        out=v_flat[r * CHUNK:(r + 1) * CHUNK, :],
                        in_=v_out.ap()[r, :].rearrange("(t c) -> t c", c=D))

                # ---- q last (overlaps the collectives) ----
                qk_proj("q", wq, True)
                qk_scales("q", sq_scale, sq_bc)
                qk_rope("q", sq_bc)

            # =========== Phase 3: attention ===========
            NKT = len(KEY_TILES)
            with tc.tile_pool(name="a_k", bufs=2) as akp, \
                 tc.tile_pool(name="a_v", bufs=2) as avp, \
                 tc.tile_pool(name="a_p", bufs=10) as app, \
                 tc.tile_pool(name="a_sb", bufs=3) as asb, \
                 tc.tile_pool(name="a_ps", bufs=3, space="PSUM") as aps, \
                 tc.tile_pool(name="acc_ps", bufs=2, space="PSUM") as accps, \
                 tc.tile_pool(name="sum_ps", bufs=2, space="PSUM") as sumps:
                for h in range(H):
                    kt_sb = akp.tile([128, T], BF16, tag="kt_sb")
                    for r in range(NC):
                        nc.sync.dma_start(
                            out=kt_sb[:, r * CHUNK:(r + 1) * CHUNK],
                            in_=k_out.ap()[r, :]
                            .rearrange("(row t) -> row t", t=CHUNK)
                            [h * 128:(h + 1) * 128, :])
                    vt_sb = avp.tile([128, NKT, 128], BF16, tag="vt_sb")
                    nc.sync.dma_start(
                        out=vt_sb[:, 0:24, :],
                        in_=v_flat.ap()[0:3072, h * 128:(h + 1) * 128]
                        .rearrange("(t p) c -> p t c", p=128))
                    nc.sync.dma_start(
                        out=vt_sb[0:48, 24:25, :],
                        in_=v_flat.ap()[3072:3120, h * 128:(h + 1) * 128]
                        .rearrange("(t p) c -> p t c", p=48))

                    acc = accps.tile([128, CHUNK], F32, tag="acc")
                    sums = sumps.tile([1, CHUNK], F32, tag="sums")
                    for kt, (k0, ksz) in enumerate(KEY_TILES):
                        sc = aps.tile([128, CHUNK], F32, tag="sc")
                        nc.tensor.matmul(
                            sc[0:ksz, :],
                            kt_sb[:, k0:k0 + ksz],
                            qT_sb[:, h * CHUNK:(h + 1) * CHUNK],
                            start=True, stop=True)
                        pr = app.tile([128, CHUNK], BF16, tag="pr")
                        nc.scalar.activation(
                            pr[0:ksz, :], sc[0:ksz, :],
                            mybir.ActivationFunctionType.Exp,
                            bias=masks_sb[0:ksz, kt:kt + 1], scale=SCALE)
                        nc.tensor.matmul(
                            acc[:, :],
                            vt_sb[0:ksz, kt, :],
                            pr[0:ksz, :],
                            start=(kt == 0), stop=(kt == NKT - 1))
                        nc.tensor.matmul(
                            sums[:, :],
                            ones_bf[0:ksz, :],
                            pr[0:ksz, :],
                            start=(kt == 0), stop=(kt == NKT - 1))
                    rec = asb.tile([1, CHUNK], F32, tag="rec")
                    nc.vector.reciprocal(rec[:, :], sums[:, :])
                    recb = asb.tile([128, CHUNK], F32, tag="recb")
                    nc.gpsimd.partition_broadcast(recb[:, :], rec[:, :])
                    nc.vector.tensor_tensor(
                        attnT_sb[:, h * CHUNK:(h + 1) * CHUNK],
                        acc[:, :], recb[:, :],
                        mybir.AluOpType.mult)
                    if debug:
                        ssb = asb.tile([1, CHUNK], F32, tag="ssb")
                        nc.vector.tensor_copy(ssb[:, :], sums[:, :])
                        nc.sync.dma_start(out=dbg_sums[h:h + 1, :], in_=ssb[:, :])
                        da = asb.tile([128, CHUNK], F32, tag="da")
                        nc.vector.tensor_copy(
                            da[:, :], attnT_sb[:, h * CHUNK:(h + 1) * CHUNK])
                        nc.sync.dma_start(
                            out=dbg_attnT[:, h * CHUNK:(h + 1) * CHUNK],
                            in_=da[:, :])

            # =========== Phase 4: o-projection ===========
            with tc.tile_pool(name="p4sb", bufs=3) as p4sb, \
                 tc.tile_pool(name="p4w", bufs=2) as p4w, \
                 tc.tile_pool(name="p4ps", bufs=3, space="PSUM") as p4ps:
                for (c0, csz) in COLG:
                    wt = p4w.tile([128, KC * 512], BF16, tag="wo_t")
                    nc.sync.dma_start(
                        out=wt[:, :].rearrange("p (c m) -> p c m", c=KC),
                        in_=wo[0:D, c0:c0 + csz]
                        .rearrange("(c p) m -> p c m", p=128))
                    for (t0, tsz) in TOK_SUBS:
                        ps = p4ps.tile([128, 512], F32, tag="o_ps")
                        for hh in range(KC):
                            nc.tensor.matmul(
                                ps[0:tsz, :],
                                attnT_sb[:, hh * CHUNK + t0:hh * CHUNK + t0 + tsz],
                                wt[:, hh * 512:(hh + 1) * 512],
                                start=(hh == 0), stop=(hh == KC - 1))
                        osb = p4sb.tile([128, 512], F32, tag="osb")
                        if apply_bias_o:
                            bob = p4sb.tile([128, 512], F32, tag="bob")
                            nc.gpsimd.partition_broadcast(
                                bob[:, :], bvo_sb[1:2, c0:c0 + csz])
                            nc.vector.tensor_tensor(
                                osb[0:tsz, :], ps[0:tsz, :], bob[0:tsz, :],
                                mybir.AluOpType.add)
                        else:
                            nc.vector.tensor_copy(osb[0:tsz, :], ps[0:tsz, :])
                        nc.sync.dma_start(out=out_part[t0:t0 + tsz, c0:c0 + csz],
                                          in_=osb[0:tsz, :])

    nc.compile()
    return nc


_NC_CACHE = {}


def _get_nc(key):
    if key not in _NC_CACHE:
        _NC_CACHE[key] = build_kernel(*key)
    return _NC_CACHE[key]


def _prep_inputs(x, freqs_cos, freqs_sin, Wq, bq, Wk, bk, Wv, bv, Wo, bo,
                 gq, gk, frame_seqlen, debug=False):
    assert int(frame_seqlen) == L
    x2d = np.asarray(x, np.float32).reshape(T, D)
    xT_full = np.ascontiguousarray(x2d.T)

    perm = np.concatenate([
        np.concatenate([np.arange(0, 128, 2), np.arange(1, 128, 2)]) + 128 * h
        for h in range(H)])
    Wqp = np.asarray(Wq, np.float32)[:, perm]
    Wkp = np.asarray(Wk, np.float32)[:, perm]
    bqp = np.asarray(bq, np.float32)[perm]
    bkp = np.asarray(bk, np.float32)[perm]
    gqp = np.asarray(gq, np.float32)[perm]
    gkp = np.asarray(gk, np.float32)[perm]

    cosT = np.asarray(freqs_cos, np.float32).T
    sinT = np.asarray(freqs_sin, np.float32).T
    costab = np.concatenate([cosT, cosT], 0)
    sintab = np.concatenate([-sinT, sinT], 0)

    frames = np.arange(T) // L
    bf16 = ml_dtypes.bfloat16

    apply_bias_qk = not (np.all(bqp == 0) and np.all(bkp == 0))
    apply_g = not (np.all(gqp == 1) and np.all(gkp == 1))
    apply_bias_v = not np.all(np.asarray(bv) == 0)
    apply_bias_o = not np.all(np.asarray(bo) == 0)
    key = (apply_bias_qk, apply_g, apply_bias_v, apply_bias_o, debug)

    shared = {
        "wq": Wqp.astype(bf16), "wk": Wkp.astype(bf16),
        "wv": np.asarray(Wv, np.float32).astype(bf16),
        "wo": np.asarray(Wo, np.float32).astype(bf16),
        "bqk2": np.concatenate([bqp, bkp]).reshape(2 * KC, 128),
        "gqk2": np.concatenate([gqp, gkp]).reshape(2 * KC, 128),
        "bvo": np.stack([np.asarray(bv, np.float32),
                         np.asarray(bo, np.float32)]),
    }
    in_maps = []
    for c in range(NC):
        t0 = c * CHUNK
        f_c = t0 // L
        m = np.where(frames <= f_c, 0.0, -30000.0).astype(np.float32)
        in_maps.append({
            **shared,
            "xT": np.ascontiguousarray(xT_full[:, t0:t0 + CHUNK]).astype(bf16),
            "cost": np.ascontiguousarray(costab[:, t0:t0 + CHUNK]),
            "sint": np.ascontiguousarray(sintab[:, t0:t0 + CHUNK]),
            "maskv": m,
        })
    return key, in_maps


def kernel(x, freqs_cos, freqs_sin, Wq, bq, Wk, bk, Wv, bv, Wo, bo,
           gq, gk, frame_seqlen):
    key, in_maps = _prep_inputs(x, freqs_cos, freqs_sin, Wq, bq, Wk, bk,
                                Wv, bv, Wo, bo, gq, gk, frame_seqlen)
    nc = _get_nc(key)
    res = run_bass_kernel_spmd(nc, in_maps, core_ids=list(range(NC)))
    out = np.empty((1, T, D), np.float32)
    for c in range(NC):
        out[0, c * CHUNK:(c + 1) * CHUNK, :] = res.results[c]["out_part"]
    return out


# revision 12
# speedup vs baseline: 1.0573x; 1.0014x over previous
"""Trainium2 Bass kernel for nn_CausalWanModel (frame-block-causal attention).

Self-contained: hardcodes shapes from the problem spec.
  B=1, T=3120, D=1536, H=12 heads, hd=128, frame_seqlen=780, 8 cores.

Sharding: sequence-parallel. Core c owns tokens [390c, 390c+390) for
q/k/v projections, attention (its queries vs all keys, block-causal mask
via per-key additive bias on the exp), and the o-projection rows. k/v are
exchanged with a single AllGather; the host gathers the 8 row-slices.

Matmuls run in bf16 (fp32 PSUM accumulation); RMSNorm statistics in fp32.
"""

import math

import numpy as np
import ml_dtypes

import concourse.bacc as bacc
import concourse.mybir as mybir
import concourse.tile as tile
from concourse.bass_utils import run_bass_kernel_spmd

F32 = mybir.dt.float32
BF16 = mybir.dt.bfloat16

NC = 8
T = 3120
D = 1536
H = 12
HD = 128
L = 780  # frame_seqlen
CHUNK = T // NC  # 390 tokens per core
KC = D // 128  # 12 contraction chunks
EPS = 1e-6
SCALE = 1.0 / math.sqrt(HD)

PADT = 512  # per-rank padded token count in the gathered v / padded key grid
KEY_TILES = []
for _r in range(NC):
    KEY_TILES += [(_r * PADT + 0, 128), (_r * PADT + 128, 128),
                  (_r * PADT + 256, 128), (_r * PADT + 384, 6)]
TOK_SUBS = [(0, 128), (128, 128), (256, 128), (384, 6)]
COLG = [(g * 512, 512) for g in range(3)]

K_ELEMS = D * CHUNK
V_ELEMS = CHUNK * D


def build_kernel(apply_bias_qk=False, apply_g=False, apply_bias_v=False,
                 apply_bias_o=False, debug=False):
    nc = bacc.Bacc("TRN2", target_bir_lowering=False, debug=False, num_devices=NC)

    # ---- I/O ----
    xT = nc.dram_tensor("xT", [D, CHUNK], BF16, kind="ExternalInput")
    wq = nc.dram_tensor("wq", [D, D], BF16, kind="ExternalInput")
    wk = nc.dram_tensor("wk", [D, D], BF16, kind="ExternalInput")
    wv = nc.dram_tensor("wv", [D, D], BF16, kind="ExternalInput")
    wo = nc.dram_tensor("wo", [D, D], BF16, kind="ExternalInput")
    cost = nc.dram_tensor("cost", [128, CHUNK], F32, kind="ExternalInput")
    sint = nc.dram_tensor("sint", [128, CHUNK], F32, kind="ExternalInput")
    maskv = nc.dram_tensor("maskv", [NC * PADT], F32, kind="ExternalInput")
    bqk2 = nc.dram_tensor("bqk2", [2 * KC, 128], F32, kind="ExternalInput")
    gqk2 = nc.dram_tensor("gqk2", [2 * KC, 128], F32, kind="ExternalInput")
    bvo = nc.dram_tensor("bvo", [2, D], F32, kind="ExternalInput")
    out_part = nc.dram_tensor("out_part", [CHUNK, D], F32, kind="ExternalOutput")

    # ---- collective buffers ----
    k_in = nc.dram_tensor("k_in", [K_ELEMS], BF16)
    v_in = nc.dram_tensor("v_in", [PADT * D], BF16)
    k_out = nc.dram_tensor("k_out", [NC, K_ELEMS], BF16, addr_space="Shared")
    v_out = nc.dram_tensor("v_out", [NC, PADT * D], BF16, addr_space="Shared")

    if debug:
        dbg_qT = nc.dram_tensor("dbg_qT", [128, KC * CHUNK], F32, kind="ExternalOutput")
        dbg_kT = nc.dram_tensor("dbg_kT", [128, KC * CHUNK], F32, kind="ExternalOutput")
        dbg_sums = nc.dram_tensor("dbg_sums", [H, CHUNK], F32, kind="ExternalOutput")
        dbg_attnT = nc.dram_tensor("dbg_attnT", [128, KC * CHUNK], F32,
                                   kind="ExternalOutput")

    kT_view = k_in.ap().rearrange("(r t) -> r t", t=CHUNK)
    v_view = v_in.ap().rearrange("(t c) -> t c", c=D)

    with tile.TileContext(nc) as tc:
        with tc.tile_pool(name="const", bufs=1) as cpool:
            xT_sb = cpool.tile([128, KC * CHUNK], BF16, tag="xT_sb")
            qT_sb = cpool.tile([128, KC * CHUNK], BF16, tag="qT_sb")
            attnT_sb = cpool.tile([128, KC * CHUNK], BF16, tag="attnT_sb")
            cost_sb = cpool.tile([128, CHUNK], F32, tag="cost_sb")
            sint_sb = cpool.tile([128, CHUNK], F32, tag="sint_sb")
            masks_sb = cpool.tile([128, len(KEY_TILES)], F32, tag="masks_sb")
            ones_f32 = cpool.tile([128, 1], F32, tag="ones_f32")
            ones_bf = cpool.tile([128, 1], BF16, tag="ones_bf")
            sq_scale = cpool.tile([1, CHUNK], F32, tag="sq_scale")
            sk_scale = cpool.tile([1, CHUNK], F32, tag="sk_scale")
            sq_bc = cpool.tile([128, CHUNK], F32, tag="sq_bc")
            sk_bc = cpool.tile([128, CHUNK], F32, tag="sk_bc")
            u_tiles = {(name, d): cpool.tile([128, CHUNK], F32,
                                             name=f"u_{name}_{d}",
                                             tag=f"u_{name}_{d}")
                       for name in ("q", "k") for d in range(KC)}

            eps_sb = cpool.tile([1, 1], F32, tag="eps_sb")
            nc.gpsimd.memset(ones_f32[:, :], 1.0)
            nc.gpsimd.memset(ones_bf[:, :], 1.0)
            nc.gpsimd.memset(eps_sb[:, :], EPS)

            for d in range(KC):
                nc.sync.dma_start(out=xT_sb[:, d * CHUNK:(d + 1) * CHUNK],
                                  in_=xT[d * 128:(d + 1) * 128, :])
            nc.sync.dma_start(out=cost_sb[:, :], in_=cost[:, :])
            nc.sync.dma_start(out=sint_sb[:, :], in_=sint[:, :])
            nc.sync.dma_start(
                out=masks_sb[:, :],
                in_=maskv.ap().rearrange("(t p) -> p t", p=128))
            bqk_sb = gqk_sb = bvo_sb = None
            if apply_bias_qk:
                bqk_sb = cpool.tile([128, 2 * KC], F32, tag="bqk_sb")
                nc.sync.dma_start(out=bqk_sb[:, :],
                                  in_=bqk2.ap().rearrange("c p -> p c"))
            if apply_g:
                gqk_sb = cpool.tile([128, 2 * KC], F32, tag="gqk_sb")
                nc.sync.dma_start(out=gqk_sb[:, :],
                                  in_=gqk2.ap().rearrange("c p -> p c"))
            if apply_bias_v or apply_bias_o:
                bvo_sb = cpool.tile([2, D], F32, tag="bvo_sb")
                nc.sync.dma_start(out=bvo_sb[:, :], in_=bvo[:, :])

            # ===== Phase 1: projections + rmsnorm + rope (k -> AG_k -> v -> AG_v -> q) =====
            with tc.tile_pool(name="p1sb", bufs=3) as p1sb, \
                 tc.tile_pool(name="p1w", bufs=3) as p1w, \
                 tc.tile_pool(name="p1ps", bufs=2, space="PSUM") as p1ps, \
                 tc.tile_pool(name="ssqps", bufs=1, space="PSUM") as ssqps:

                ssq_ps = {}

                def qk_proj(name, w, is_q):
                    ssq_ps[name] = ssqps.tile([1, CHUNK], F32, name=f"ssq_{name}",
                                              tag=f"ssq_{name}")
                    for d in range(KC):
                        wt = p1w.tile([128, D], BF16, tag="wqk_t", name="wqk_t")
                        nc.sync.dma_start(
                            out=wt[:, :].rearrange("p (c m) -> p c m", c=KC),
                            in_=w[0:D, d * 128:(d + 1) * 128]
                            .rearrange("(c p) m -> p c m", p=128))
                        ps = p1ps.tile([128, CHUNK], F32, tag="proj_ps",
                                       name="proj_ps")
                        for c in range(KC):
                            nc.tensor.matmul(
                                ps[:, :],
                                wt[:, c * 128:(c + 1) * 128],
                                xT_sb[:, c * CHUNK:(c + 1) * CHUNK],
                                start=(c == 0), stop=(c == KC - 1))
                        ur = u_tiles[(name, d)]
                        if apply_bias_qk:
                            bias_col = (0 if is_q else KC) + d
                            nc.vector.tensor_scalar_add(
                                ur[:, :], ps[:, :], bqk_sb[:, bias_col:bias_col + 1])
                        else:
                            nc.vector.tensor_copy(ur[:, :], ps[:, :])
                        sq = p1sb.tile([128, CHUNK], BF16, tag="sqsb", name="sqsb")
                        nc.vector.tensor_tensor(sq[:, :], ur[:, :], ur[:, :],
                                                mybir.AluOpType.mult)
                        nc.tensor.matmul(ssq_ps[name][:, :], ones_bf[:, :], sq[:, :],
                                         start=(d == 0), stop=(d == KC - 1))

                def qk_scales(name, stile, sbc):
                    nc.scalar.activation(stile[:, :], ssq_ps[name][:, :],
                                         mybir.ActivationFunctionType.Sqrt,
                                         bias=eps_sb[:, :], scale=1.0 / D)
                    nc.vector.reciprocal(stile[:, :], stile[:, :])
                    nc.gpsimd.partition_broadcast(sbc[:, :], stile[:, :])

                def qk_rope(name, sbc):
                    for d in range(KC):
                        ur = u_tiles[(name, d)]
                        qs = p1sb.tile([128, CHUNK], F32, tag="qs", name="qs")
                        nc.vector.tensor_tensor(
                            qs[:, :], ur[:, :], sbc[:, :],
                            mybir.AluOpType.mult)
                        if apply_g:
                            gcol = (0 if name == "q" else KC) + d
                            nc.vector.tensor_scalar_mul(
                                qs[:, :], qs[:, :], gqk_sb[:, gcol:gcol + 1])
                        qsw = p1sb.tile([128, CHUNK], F32, tag="qsw", name="qsw")
                        nc.scalar.dma_start(out=qsw[0:64, :], in_=qs[64:128, :])
                        nc.scalar.dma_start(out=qsw[64:128, :], in_=qs[0:64, :])
                        t1 = p1sb.tile([128, CHUNK], F32, tag="rope_t1", name="rope_t1")
                        t2 = p1sb.tile([128, CHUNK], F32, tag="rope_t2", name="rope_t2")
                        nc.vector.tensor_tensor(t1[:, :], qs[:, :], cost_sb[:, :],
                                                mybir.AluOpType.mult)
                        nc.vector.tensor_tensor(t2[:, :], qsw[:, :], sint_sb[:, :],
                                                mybir.AluOpType.mult)
                        if name == "q":
                            dst = qT_sb[:, d * CHUNK:(d + 1) * CHUNK]
                            nc.vector.tensor_tensor(dst[:, :], t1[:, :], t2[:, :],
                                                    mybir.AluOpType.add)
                            if debug:
                                df = p1sb.tile([128, CHUNK], F32, tag="dbgf",
                                               name="dbgf")
                                nc.vector.tensor_copy(df[:, :], dst)
                                nc.sync.dma_start(
                                    out=dbg_qT[:, d * CHUNK:(d + 1) * CHUNK],
                                    in_=df[:, :])
                        else:
                            kr = p1sb.tile([128, CHUNK], BF16, tag="krope",
                                           name="krope")
                            nc.vector.tensor_tensor(kr[:, :], t1[:, :], t2[:, :],
                                                    mybir.AluOpType.add)
                            nc.sync.dma_start(
                                out=kT_view[d * 128:(d + 1) * 128, :],
                                in_=kr[:, :])
                            if debug:
                                df = p1sb.tile([128, CHUNK], F32, tag="dbgf",
                                               name="dbgf")
                                nc.vector.tensor_copy(df[:, :], kr[:, :])
                                nc.sync.dma_start(
                                    out=dbg_kT[:, d * CHUNK:(d + 1) * CHUNK],
                                    in_=df[:, :])

                # ---- k first, then AG_k ----
                qk_proj("k", wk, False)
                qk_scales("k", sk_scale, sk_bc)
                qk_rope("k", sk_bc)
                nc.gpsimd.collective_compute(
                    "AllGather", mybir.AluOpType.bypass,
                    ins=[k_in.ap().opt()],
                    outs=[k_out.ap().opt()],
                    replica_groups=[list(range(NC))],
                )

                # ---- v projection, then AG_v ----
                for (c0, csz) in COLG:
                    wt = p1w.tile([128, KC * 512], BF16, tag="wv_t", name="wv_t")
                    nc.sync.dma_start(
                        out=wt[:, :].rearrange("p (c m) -> p c m", c=KC),
                        in_=wv[0:D, c0:c0 + csz]
                        .rearrange("(c p) m -> p c m", p=128))
                    for (t0, tsz) in TOK_SUBS:
                        ps = p1ps.tile([128, 512], F32, tag="v_ps", name="v_ps")
                        for c in range(KC):
                            nc.tensor.matmul(
                                ps[0:tsz, :],
                                xT_sb[:, c * CHUNK + t0:c * CHUNK + t0 + tsz],
                                wt[:, c * 512:(c + 1) * 512],
                                start=(c == 0), stop=(c == KC - 1))
                        vsb = p1sb.tile([128, 512], BF16, tag="vsb", name="vsb")
                        if apply_bias_v:
                            bvb = p1sb.tile([128, 512], F32, tag="bvb", name="bvb")
                            nc.gpsimd.partition_broadcast(
                                bvb[:, :], bvo_sb[0:1, c0:c0 + csz])
                            nc.vector.tensor_tensor(
                                vsb[0:tsz, :], ps[0:tsz, :], bvb[0:tsz, :],
                                mybir.AluOpType.add)
                        else:
                            nc.vector.tensor_copy(vsb[0:tsz, :], ps[0:tsz, :])
                        nc.sync.dma_start(out=v_view[t0:t0 + tsz, c0:c0 + csz],
                                          in_=vsb[0:tsz, :])
                nc.gpsimd.collective_compute(
                    "AllGather", mybir.AluOpType.bypass,
                    ins=[v_in.ap().opt()],
                    outs=[v_out.ap().opt()],
                    replica_groups=[list(range(NC))],
                )
                # ---- q last (overlaps the collectives) ----
                qk_proj("q", wq, True)
                qk_scales("q", sq_scale, sq_bc)
                qk_rope("q", sq_bc)

            # =========== Phase 3: attention ===========
            NKT = len(KEY_TILES)
            with tc.tile_pool(name="a_k", bufs=2) as akp, \
                 tc.tile_pool(name="a_v", bufs=2) as avp, \
                 tc.tile_pool(name="a_p", bufs=10) as app, \
                 tc.tile_pool(name="a_sb", bufs=3) as asb, \
                 tc.tile_pool(name="a_ps", bufs=3, space="PSUM") as aps, \
                 tc.tile_pool(name="acc_ps", bufs=2, space="PSUM") as accps, \
                 tc.tile_pool(name="sum_ps", bufs=2, space="PSUM") as sumps:
                for h in range(H):
                    kt_sb = akp.tile([128, NC * PADT], BF16, tag="kt_sb")
                    for r in range(NC):
                        nc.sync.dma_start(
                            out=kt_sb[:, r * PADT:r * PADT + CHUNK],
                            in_=k_out.ap()[r, :]
                            .rearrange("(row t) -> row t", t=CHUNK)
                            [h * 128:(h + 1) * 128, :])
                    vt_sb = avp.tile([128, NKT, 128], BF16, tag="vt_sb")
                    nc.sync.dma_start(
                        out=vt_sb[:, :, :],
                        in_=v_out.ap().rearrange("r (t c) -> (r t) c", c=D)
                        [:, h * 128:(h + 1) * 128]
                        .rearrange("(t p) c -> p t c", p=128))

                    acc = accps.tile([128, CHUNK], F32, tag="acc")
                    sums = sumps.tile([1, CHUNK], F32, tag="sums")
                    for kt, (k0, ksz) in enumerate(KEY_TILES):
                        sc = aps.tile([128, CHUNK], F32, tag="sc")
                        nc.tensor.matmul(
                            sc[0:ksz, :],
                            kt_sb[:, k0:k0 + ksz],
                            qT_sb[:, h * CHUNK:(h + 1) * CHUNK],
                            start=True, stop=True)
                        pr = app.tile([128, CHUNK], BF16, tag="pr")
                        nc.scalar.activation(
                            pr[0:ksz, :], sc[0:ksz, :],
                            mybir.ActivationFunctionType.Exp,
                            bias=masks_sb[0:ksz, kt:kt + 1], scale=SCALE)
                        nc.tensor.matmul(
                            acc[:, :],
                            vt_sb[0:ksz, kt, :],
                            pr[0:ksz, :],
                            start=(kt == 0), stop=(kt == NKT - 1))
                        nc.tensor.matmul(
                            sums[:, :],
                            ones_bf[0:ksz, :],
                            pr[0:ksz, :],
                            start=(kt == 0), stop=(kt == NKT - 1))
                    rec = asb.tile([1, CHUNK], F32, tag="rec")
                    nc.vector.reciprocal(rec[:, :], sums[:, :])
                    recb = asb.tile([128, CHUNK], F32, tag="recb")
                    nc.gpsimd.partition_broadcast(recb[:, :], rec[:, :])
                    nc.vector.tensor_tensor(
                        attnT_sb[:, h * CHUNK:(h + 1) * CHUNK],
                        acc[:, :], recb[:, :],
                        mybir.AluOpType.mult)
                    if debug:
                        ssb = asb.tile([1, CHUNK], F32, tag="ssb")
                        nc.vector.tensor_copy(ssb[:, :], sums[:, :])
                        nc.sync.dma_start(out=dbg_sums[h:h + 1, :], in_=ssb[:, :])
                        da = asb.tile([128, CHUNK], F32, tag="da")
                        nc.vector.tensor_copy(
                            da[:, :], attnT_sb[:, h * CHUNK:(h + 1) * CHUNK])
                        nc.sync.dma_start(
                            out=dbg_attnT[:, h * CHUNK:(h + 1) * CHUNK],
                            in_=da[:, :])

            # =========== Phase 4: o-projection ===========
            with tc.tile_pool(name="p4sb", bufs=3) as p4sb, \
                 tc.tile_pool(name="p4w", bufs=2) as p4w, \
                 tc.tile_pool(name="p4ps", bufs=3, space="PSUM") as p4ps:
                for (c0, csz) in COLG:
                    wt = p4w.tile([128, KC * 512], BF16, tag="wo_t")
                    nc.sync.dma_start(
                        out=wt[:, :].rearrange("p (c m) -> p c m", c=KC),
                        in_=wo[0:D, c0:c0 + csz]
                        .rearrange("(c p) m -> p c m", p=128))
                    for (t0, tsz) in TOK_SUBS:
                        ps = p4ps.tile([128, 512], F32, tag="o_ps")
                        for hh in range(KC):
                            nc.tensor.matmul(
                                ps[0:tsz, :],
                                attnT_sb[:, hh * CHUNK + t0:hh * CHUNK + t0 + tsz],
                                wt[:, hh * 512:(hh + 1) * 512],
                                start=(hh == 0), stop=(hh == KC - 1))
                        osb = p4sb.tile([128, 512], F32, tag="osb")
                        if apply_bias_o:
                            bob = p4sb.tile([128, 512], F32, tag="bob")
                            nc.gpsimd.partition_broadcast(
                                bob[:, :], bvo_sb[1:2, c0:c0 + csz])
                            nc.vector.tensor_tensor(
                                osb[0:tsz, :], ps[0:tsz, :], bob[0:tsz, :],
                                mybir.AluOpType.add)
                        else:
                            nc.vector.tensor_copy(osb[0:tsz, :], ps[0:tsz, :])
                        nc.sync.dma_start(out=out_part[t0:t0 + tsz, c0:c0 + csz],
                                          in_=osb[0:tsz, :])

    nc.compile()
    return nc


_NC_CACHE = {}


def _get_nc(key):
    if key not in _NC_CACHE:
        _NC_CACHE[key] = build_kernel(*key)
    return _NC_CACHE[key]


def _prep_inputs(x, freqs_cos, freqs_sin, Wq, bq, Wk, bk, Wv, bv, Wo, bo,
                 gq, gk, frame_seqlen, debug=False):
    assert int(frame_seqlen) == L
    x2d = np.asarray(x, np.float32).reshape(T, D)
    xT_full = np.ascontiguousarray(x2d.T)

    perm = np.concatenate([
        np.concatenate([np.arange(0, 128, 2), np.arange(1, 128, 2)]) + 128 * h
        for h in range(H)])
    Wqp = np.asarray(Wq, np.float32)[:, perm]
    Wkp = np.asarray(Wk, np.float32)[:, perm]
    bqp = np.asarray(bq, np.float32)[perm]
    bkp = np.asarray(bk, np.float32)[perm]
    gqp = np.asarray(gq, np.float32)[perm]
    gkp = np.asarray(gk, np.float32)[perm]

    cosT = np.asarray(freqs_cos, np.float32).T
    sinT = np.asarray(freqs_sin, np.float32).T
    costab = np.concatenate([cosT, cosT], 0)
    sintab = np.concatenate([-sinT, sinT], 0)

    frames = np.arange(T) // L
    bf16 = ml_dtypes.bfloat16

    apply_bias_qk = not (np.all(bqp == 0) and np.all(bkp == 0))
    apply_g = not (np.all(gqp == 1) and np.all(gkp == 1))
    apply_bias_v = not np.all(np.asarray(bv) == 0)
    apply_bias_o = not np.all(np.asarray(bo) == 0)
    key = (apply_bias_qk, apply_g, apply_bias_v, apply_bias_o, debug)

    shared = {
        "wq": Wqp.astype(bf16), "wk": Wkp.astype(bf16),
        "wv": np.asarray(Wv, np.float32).astype(bf16),
        "wo": np.asarray(Wo, np.float32).astype(bf16),
        "bqk2": np.concatenate([bqp, bkp]).reshape(2 * KC, 128),
        "gqk2": np.concatenate([gqp, gkp]).reshape(2 * KC, 128),
        "bvo": np.stack([np.asarray(bv, np.float32),
                         np.asarray(bo, np.float32)]),
    }
    in_maps = []
    for c in range(NC):
        t0 = c * CHUNK
        f_c = t0 // L
        m = np.where(frames <= f_c, 0.0, -30000.0).astype(np.float32)
        mpad = np.full(NC * 512, -30000.0, np.float32)
        for r in range(NC):
            mpad[r * 512:r * 512 + CHUNK] = m[r * CHUNK:(r + 1) * CHUNK]
        in_maps.append({
            **shared,
            "xT": np.ascontiguousarray(xT_full[:, t0:t0 + CHUNK]).astype(bf16),
            "cost": np.ascontiguousarray(costab[:, t0:t0 + CHUNK]),
            "sint": np.ascontiguousarray(sintab[:, t0:t0 + CHUNK]),
            "maskv": mpad,
        })
    return key, in_maps


def kernel(x, freqs_cos, freqs_sin, Wq, bq, Wk, bk, Wv, bv, Wo, bo,
           gq, gk, frame_seqlen):
    key, in_maps = _prep_inputs(x, freqs_cos, freqs_sin, Wq, bq, Wk, bk,
                                Wv, bv, Wo, bo, gq, gk, frame_seqlen)
    nc = _get_nc(key)
    res = run_bass_kernel_spmd(nc, in_maps, core_ids=list(range(NC)))
    out = np.empty((1, T, D), np.float32)
    for c in range(NC):
        out[0, c * CHUNK:(c + 1) * CHUNK, :] = res.results[c]["out_part"]
    return out
